# revision 34
# baseline (speedup 1.0000x reference)
"""Trainium2 Bass kernel for ActivationGATSingleHeadLayer (GNN message passing).

Reference computation (jax):
    e = relu(sum(z[src] * z[dst], -1))             # [E]
    alpha = segment_softmax(e, dst)                # two-pass in ref
    h = segment_sum(alpha[:, None] * z[src], dst)  # [N, D]
    out = relu(batchnorm(h))                       # training-mode stats

Strategy (8 NeuronCores):
  * Host shards edges by dst range: core c owns dst in [c*NPC, (c+1)*NPC).
    All segment reductions are core-local; the only collective is an
    AllReduce of 128 floats of BatchNorm statistics.
  * Segment softmax is collapsed to one pass:
        h[n] = sum_e w_e * z[src_e] / sum_e w_e,  w_e = exp(relu(e_e) - SHIFT)
    The constant SHIFT (=64) replaces the segment max: relu makes e >= 0 and
    e <= max ||z_i||^2 ~ chi2_64 stays far below SHIFT + 88, so exp never
    overflows and the result is mathematically identical.
  * Edges are sorted by dst and grouped into 128-node windows. Per 128-edge
    tile, one-hot membership matrices are built arithmetically:
        onehotT[n, e] = relu(1 - (dstrel_e - n)^2)     (Scalar engine, 2 ops)
        onehot[e, n]  = (iota[n] == dstrel_e)          (Vector engine, 1 op)
  * z[dst] rows are NOT gathered: they are expanded on the TensorEngine as
    psum_zd = onehotT^T @ zwin (f32, an exact row selection). This halves the
    SWDGE descriptor-generation load, which is the dominant cost (the Q7
    ucode spends ~8ns per gather index).
  * Aggregation also runs on the TensorEngine (HBM scatter-add races on
    duplicate indices): psum[win] += onehot^T @ [w*z_src | w] in bf16
    (values are linearly averaged, so bf16 rounding stays ~1e-3).
  * z[src] rows are fetched with SWDGE dma_gather (f32, 256B elements);
    int16 indices only reach 32767, so tiles are segregated into lo
    (src < SPLIT) / hi sections gathering from the two z-table halves.
    SWDGE ops are capped at 1024 indices (larger ops overflow the
    descriptor-ring carveout and hang the device).
  * h is stored feature-major [128, D, NW] so BatchNorm stats reduce over
    contiguous memory; stats cross partitions via one matmul against ones,
    AllReduce of 128 floats, partition-broadcast back, normalize + relu.
"""

import sys

for _p in ("/opt/trn_rl_repo", "/root/.axon_site/_ro/trn_rl_repo"):
    if _p not in sys.path:
        sys.path.append(_p)

import ml_dtypes
import numpy as np

# ---------------------------------------------------------------- geometry
N_NODES = 50000
N_EDGES = 800000
D = 64
NCORES = 8

EPS = 1e-5          # BatchNorm eps (matches reference)
TINY = 1e-30        # denom guard for isolated nodes
SHIFT = 64.0        # constant subtracted inside exp
WIN = 128           # nodes per aggregation window (= PSUM partition dim)
MAX_PSUM_WIN = 5    # windows per phase: 1 bank each + 2 banks zdst + 1 stats
CHUNK_TILES = 8     # tiles per gather chunk; SWDGE ops above ~1024 indices
                    # overflow the descriptor-ring carveout and hang


def _derive(n_nodes, split):
    npc = n_nodes // NCORES
    nw = -(-npc // WIN)
    return dict(
        n_nodes=n_nodes,
        npc=npc,
        nw=nw,
        h_rows=nw * WIN,
        split=split,
    )


CFG = _derive(N_NODES, split=25000)


# ---------------------------------------------------------------- host prep
def _wrap_tile_idx(arr):
    """[T, 128] int -> [128, T, 8] int16 SWDGE layout, partition-major:
    within-tile edge p at [p%16 (+16g), tile, p//16]."""
    t = arr.shape[0]
    w = arr.reshape(t, 8, 16).transpose(0, 2, 1).astype(np.int16)  # [T,16,8]
    w = np.tile(w, (1, 8, 1))                                      # [T,128,8]
    return w.transpose(1, 0, 2).copy()                             # [128,T,8]


def prep_inputs(z, src, dst, gamma, beta, cfg=CFG):
    """Shard edges by dst range, sort by dst, build window/tile plan.

    Returns (in_maps, plan). The plan (tile metadata, section table kinds,
    chunking) is identical across cores, as SPMD requires.
    """
    z = np.ascontiguousarray(np.asarray(z, dtype=np.float32))
    src = np.asarray(src).astype(np.int64)
    dst = np.asarray(dst).astype(np.int64)
    gamma = np.asarray(gamma, dtype=np.float32)
    beta = np.asarray(beta, dtype=np.float32)

    npc, split, nw = cfg["npc"], cfg["split"], cfg["nw"]
    h_rows = cfg["h_rows"]

    # per-core, per-window, per-type edge lists
    edges = [[[None, None] for _ in range(nw)] for _ in range(NCORES)]
    core_of = dst // npc
    for c in range(NCORES):
        m = core_of == c
        s, ld = src[m], dst[m] - c * npc
        order = np.argsort(ld, kind="stable")
        s, ld = s[order], ld[order]
        w_of = ld // WIN
        lo = s < split
        for w in range(nw):
            wm = w_of == w
            edges[c][w][0] = (s[wm & lo], ld[wm & lo])
            edges[c][w][1] = (s[wm & ~lo] - split, ld[wm & ~lo])

    # equalized tile counts (identical across cores)
    nt = np.zeros((2, nw), dtype=np.int64)
    for ty in range(2):
        for w in range(nw):
            mx = max(len(edges[c][w][ty][0]) for c in range(NCORES))
            nt[ty, w] = -(-mx // 128)
    nt[0] = np.maximum(nt[0], 1)  # every window needs >= 1 tile (PSUM init)

    # phases / sections / global tile order
    phases = [list(range(i, min(i + MAX_PSUM_WIN, nw)))
              for i in range(0, nw, MAX_PSUM_WIN)]
    sections = []   # (ty, [(w, local_tile_j), ...]) in global tile order
    tile_meta = []  # (window, start, stop) per global tile
    for ph in phases:
        for ty in range(2):
            tl = []
            for w in ph:
                for j in range(nt[ty, w]):
                    start = ty == 0 and j == 0
                    stop = (
                        (ty == 1 and j == nt[1, w] - 1)
                        if nt[1, w] > 0
                        else (ty == 0 and j == nt[0, w] - 1)
                    )
                    tile_meta.append((w, start, stop))
                    tl.append((w, j))
            if tl:
                sections.append((ty, tl))

    t_total = len(tile_meta)
    plan = dict(
        cfg=cfg,
        nt=nt,
        phases=phases,
        sections=sections,
        tile_meta=tile_meta,
        t_total=t_total,
    )

    gb = np.stack([gamma, beta]).astype(np.float32)
    iota = np.arange(128, dtype=np.float32)
    niota = -iota

    in_maps = []
    for c in range(NCORES):
        isrc = np.zeros((t_total, 128), dtype=np.int64)
        drel = np.full((t_total, 128), -1.0, dtype=np.float32)
        g = 0
        for ty, tl in sections:
            for w, j in tl:
                s, ld = edges[c][w][ty]
                seg_s = s[j * 128 : (j + 1) * 128]
                seg_d = ld[j * 128 : (j + 1) * 128]
                k = len(seg_s)
                isrc[g, :k] = seg_s
                drel[g, :k] = (seg_d - w * WIN).astype(np.float32)
                g += 1
        assert g == t_total

        zs = np.zeros((h_rows, D), dtype=np.float32)
        zs[:npc] = z[c * npc : (c + 1) * npc]

        in_maps.append(
            {
                "z": z,
                "zs": zs,
                "isrc": _wrap_tile_idx(isrc),
                "drel": drel.T.copy(),          # [128, T] edge-partition-major
                "drelf": drel.reshape(-1).astype(ml_dtypes.bfloat16),
                "gb": gb,
                "iota": iota,
                "niota": niota,
            }
        )
    return in_maps, plan


# ---------------------------------------------------------------- device graph
def build_nc(plan, n_total_nodes=None):
    """Build the SPMD Bass graph (identical on all cores)."""
    from concourse import bacc, tile
    from concourse.bass import mybir

    f32 = mybir.dt.float32
    bf16 = mybir.dt.bfloat16
    i16 = mybir.dt.int16
    AX = mybir.AxisListType
    ALU = mybir.AluOpType
    ACTF = mybir.ActivationFunctionType

    cfg = plan["cfg"]
    nw, split = cfg["nw"], cfg["split"]
    h_rows, n_nodes = cfg["h_rows"], cfg["n_nodes"]
    if n_total_nodes is None:
        n_total_nodes = n_nodes
    t_total = plan["t_total"]
    tile_meta = plan["tile_meta"]

    nc = bacc.Bacc(
        "TRN2",
        target_bir_lowering=False,
        debug=False,
        num_devices=NCORES,
        num_swdge_queues=1,
    )

    z_d = nc.dram_tensor("z", [n_nodes, D], f32, kind="ExternalInput")
    zs_d = nc.dram_tensor("zs", [h_rows, D], f32, kind="ExternalInput")
    isrc_d = nc.dram_tensor("isrc", [128, t_total, 8], i16, kind="ExternalInput")
    drel_d = nc.dram_tensor("drel", [128, t_total], f32, kind="ExternalInput")
    drelf_d = nc.dram_tensor("drelf", [t_total * 128], bf16, kind="ExternalInput")
    gb_d = nc.dram_tensor("gb", [2, D], f32, kind="ExternalInput")
    iota_d = nc.dram_tensor("iota", [128], f32, kind="ExternalInput")
    niota_d = nc.dram_tensor("niota", [128], f32, kind="ExternalInput")
    out_d = nc.dram_tensor("out", [h_rows, D], f32, kind="ExternalOutput")

    CT = CHUNK_TILES

    with tile.TileContext(nc) as tc:
        with (
            tc.tile_pool(name="const", bufs=1) as constp,
            tc.tile_pool(name="data", bufs=7) as datap,
            tc.tile_pool(name="oh", bufs=6) as ohp,
            tc.tile_pool(name="small", bufs=6) as smallp,
            tc.tile_pool(name="fin", bufs=1) as finp,
            tc.tile_pool(name="dram", bufs=1, space="DRAM") as dramp,
            tc.tile_pool(name="psum", bufs=1, space="PSUM") as psump,
            tc.tile_pool(name="psz", bufs=3, space="PSUM") as pszp,
        ):
            shiftb = constp.tile([128, 1], f32)
            nc.vector.memset(shiftb[:], -SHIFT)
            epsb = constp.tile([128, 1], f32)
            nc.vector.memset(epsb[:], EPS)
            ones = constp.tile([128, 1], f32)
            nc.vector.memset(ones[:], 1.0)
            iotat = constp.tile([128, 128], f32)
            nc.sync.dma_start(iotat[:], iota_d.ap().partition_broadcast(128))
            niotac = constp.tile([128, 1], f32)
            nc.sync.dma_start(niotac[:], niota_d.ap().unsqueeze(1))

            # preload all SWDGE indices + dstrel (partition-major layouts)
            isrc_sb = constp.tile([128, t_total, 8], i16)
            nc.sync.dma_start(isrc_sb[:], isrc_d[:, :, :])
            drel_sb = constp.tile([128, t_total], f32)
            nc.sync.dma_start(drel_sb[:], drel_d[:, :])

            h_all = finp.tile([128, D, nw], f32)

            g = 0  # global tile cursor
            for ph in plan["phases"]:
                nwp = len(ph)
                w0 = ph[0]
                psb = psump.tile([128, MAX_PSUM_WIN, 512], f32, tag="psb")
                zwin = datap.tile([128, MAX_PSUM_WIN, D], f32, tag="zwin")
                nc.sync.dma_start(
                    zwin[:, 0:nwp, :],
                    zs_d[w0 * WIN : (w0 + nwp) * WIN, :].rearrange(
                        "(w p) d -> p w d", p=128
                    ),
                )
                ph_secs = [
                    (ty, tl) for (ty, tl) in plan["sections"] if tl[0][0] in ph
                ]
                for ty, tl in ph_secs:
                    table = z_d[0:split, :] if ty == 0 else z_d[split:n_nodes, :]
                    for c0 in range(0, len(tl), CT):
                        ct = min(CT, len(tl) - c0)
                        t0 = g + c0
                        ne = ct * 128

                        zsrc = datap.tile([128, CT, D], f32, tag="zsrc")
                        nc.gpsimd.dma_gather(
                            zsrc[:, 0:ct, :],
                            table,
                            isrc_sb[:, t0 : t0 + ct, :],
                            ne,
                            ne,
                            D,
                        )

                        # transposed one-hot on the Scalar engine:
                        # ohT[n, e] = relu(1 - (drel_e - n)^2)
                        drbc = ohp.tile([128, CT * 128], bf16, tag="drbc")
                        nc.sync.dma_start(
                            drbc[:, 0:ne],
                            drelf_d[t0 * 128 : t0 * 128 + ne].partition_broadcast(
                                128
                            ),
                        )
                        ohT = ohp.tile([128, CT * 128], f32, tag="ohT")
                        nc.scalar.activation(
                            ohT[:, 0:ne], drbc[:, 0:ne], ACTF.Square,
                            bias=niotac[:], scale=1.0,
                        )
                        nc.scalar.activation(
                            ohT[:, 0:ne], ohT[:, 0:ne], ACTF.Relu,
                            bias=ones[:], scale=-1.0,
                        )

                        # z[dst] expansion: psum_zd[:, t, :] = ohT_t^T @ zwin_t
                        pzd = pszp.tile([128, CT, D], f32, tag="zd")
                        for tl_i in range(ct):
                            win = tile_meta[t0 + tl_i][0]
                            nc.tensor.matmul(
                                pzd[:, tl_i, :],
                                ohT[:, tl_i * 128 : (tl_i + 1) * 128],
                                zwin[:, win - w0, :],
                                start=tl_i == 0,
                                stop=tl_i == ct - 1,
                            )

                        # edge scores and weights
                        prod = datap.tile([128, CT, D], f32, tag="prod")
                        e = smallp.tile([128, CT], f32, tag="e")
                        wt = smallp.tile([128, CT], f32, tag="wt")
                        nc.vector.tensor_mul(
                            prod[:, 0:ct, :], zsrc[:, 0:ct, :], pzd[:, 0:ct, :]
                        )
                        nc.vector.tensor_reduce(
                            e[:, 0:ct], prod[:, 0:ct, :], axis=AX.X, op=ALU.add
                        )
                        nc.vector.tensor_scalar_max(e[:, 0:ct], e[:, 0:ct], 0.0)
                        nc.scalar.activation(
                            wt[:, 0:ct], e[:, 0:ct], ACTF.Exp,
                            bias=shiftb[:], scale=1.0,
                        )

                        # vals = [w * z_src | w] in bf16
                        vals = datap.tile([128, CT, D + 1], bf16, tag="vals")
                        nc.scalar.copy(vals[:, 0:ct, D], wt[:, 0:ct])
                        nc.vector.tensor_mul(
                            vals[:, 0:ct, 0:D],
                            zsrc[:, 0:ct, :],
                            wt[:, 0:ct].unsqueeze(2).broadcast_to((128, ct, D)),
                        )

                        # aggregation one-hot (edge-major) in bf16
                        oh = ohp.tile([128, CT, 128], bf16, tag="oh")
                        nc.vector.tensor_tensor(
                            oh[:, 0:ct, :],
                            iotat[:].unsqueeze(1).broadcast_to((128, ct, 128)),
                            drel_sb[:, t0 : t0 + ct]
                            .unsqueeze(2)
                            .broadcast_to((128, ct, 128)),
                            op=ALU.is_equal,
                        )

                        for tl_i in range(ct):
                            win, st, sp = tile_meta[t0 + tl_i]
                            slot = win - w0
                            nc.tensor.matmul(
                                psb[:, slot, 0 : D + 1],
                                oh[:, tl_i, :],
                                vals[:, tl_i, :],
                                start=st,
                                stop=sp,
                            )
                    g += len(tl)

                # drain phase: h = num / denom, written feature-major
                denp = smallp.tile([128, MAX_PSUM_WIN], f32, tag="den")
                recp = smallp.tile([128, MAX_PSUM_WIN], f32, tag="rec")
                nc.vector.tensor_scalar_add(
                    denp[:, 0:nwp], psb[:, 0:nwp, D], TINY
                )
                nc.vector.reciprocal(recp[:, 0:nwp], denp[:, 0:nwp])
                nc.vector.tensor_mul(
                    h_all[:, :, w0 : w0 + nwp].transpose((0, 2, 1)),
                    psb[:, 0:nwp, 0:D],
                    recp[:, 0:nwp].unsqueeze(2).broadcast_to((128, nwp, D)),
                )

            # ---- BatchNorm stats: s1 = sum(h), s2 = sum(h^2) over all nodes
            hsq = finp.tile([128, D, nw], f32, tag="hsq")
            nc.scalar.square(hsq[:], h_all[:])

            stats = smallp.tile([128, 2 * D], f32, tag="stats")
            nc.vector.tensor_reduce(
                stats[:, 0:D], h_all[:], axis=AX.X, op=ALU.add
            )
            nc.vector.tensor_reduce(
                stats[:, D : 2 * D], hsq[:], axis=AX.X, op=ALU.add
            )

            ps = pszp.tile([1, 2 * D], f32, tag="zd")
            nc.tensor.matmul(ps[:], ones[:], stats[:], start=True, stop=True)
            srow = smallp.tile([1, 2 * D], f32, tag="srow")
            nc.scalar.copy(srow[:], ps[:])

            cc_in = dramp.tile([1, 2 * D], f32)
            cc_out = dramp.tile([1, 2 * D], f32)
            nc.sync.dma_start(cc_in[:], srow[:])
            nc.gpsimd.collective_compute(
                "AllReduce",
                ALU.add,
                ins=[cc_in.opt()],
                outs=[cc_out.opt()],
                replica_groups=[list(range(NCORES))],
            )

            G = smallp.tile([128, 2 * D], f32, tag="G")
            nc.sync.dma_start(G[:], cc_out[:].squeeze(0).partition_broadcast(128))
            gbB = constp.tile([128, 2 * D], f32)
            nc.sync.dma_start(gbB[:], gb_d.ap().flatten().partition_broadcast(128))

            inv_n = 1.0 / float(n_total_nodes)
            mean = smallp.tile([128, D], f32, tag="mean")
            var = smallp.tile([128, D], f32, tag="var")
            nc.scalar.mul(mean[:], G[:, 0:D], inv_n)
            nc.scalar.mul(var[:], G[:, D : 2 * D], inv_n)
            msq = smallp.tile([128, D], f32, tag="msq")
            nc.vector.tensor_mul(msq[:], mean[:], mean[:])
            nc.vector.tensor_sub(var[:], var[:], msq[:])
            std = smallp.tile([128, D], f32, tag="std")
            nc.scalar.activation(std[:], var[:], ACTF.Sqrt, bias=epsb[:], scale=1.0)
            rstd = smallp.tile([128, D], f32, tag="rstd")
            nc.vector.reciprocal(rstd[:], std[:])

            a = smallp.tile([128, D], f32, tag="a")
            b = smallp.tile([128, D], f32, tag="b")
            nc.vector.tensor_mul(a[:], gbB[:, 0:D], rstd[:])
            nc.vector.tensor_mul(b[:], mean[:], a[:])
            nc.vector.tensor_sub(b[:], gbB[:, D : 2 * D], b[:])

            # y stored node-major so the output DMA gets 256B-contiguous runs
            y = finp.tile([128, nw, D], f32, tag="hsq")
            nc.vector.tensor_mul(
                y[:],
                h_all[:].transpose((0, 2, 1)),
                a[:].unsqueeze(1).broadcast_to((128, nw, D)),
            )
            nc.vector.tensor_add(
                y[:], y[:], b[:].unsqueeze(1).broadcast_to((128, nw, D))
            )
            nc.vector.tensor_relu(y[:], y[:])

            outv = out_d.ap().rearrange("(c p) f -> p c f", p=128)
            nc.sync.dma_start(outv, y[:])

    nc.compile()
    return nc


# ---------------------------------------------------------------- entry point
TRACE = False          # set True by test harnesses to capture exec_time_ns
LAST_RESULT = None     # BassKernelResults of the most recent kernel() call


def kernel(**inputs):
    z = inputs["z"]
    src = inputs["src"]
    dst = inputs["dst"]
    gamma = inputs["gamma"]
    beta = inputs["beta"]

    from concourse.bass_utils import run_bass_kernel_spmd

    in_maps, plan = prep_inputs(z, src, dst, gamma, beta)
    nc = build_nc(plan)
    res = run_bass_kernel_spmd(
        nc, in_maps, core_ids=list(range(NCORES)), trace=TRACE
    )
    global LAST_RESULT
    LAST_RESULT = res

    npc = CFG["npc"]
    out = np.empty((N_NODES, D), dtype=np.float32)
    for c in range(NCORES):
        out[c * npc : (c + 1) * npc] = res.results[c]["out"][:npc]
    return out


# revision 35
# speedup vs baseline: 1.2826x; 1.2826x over previous
"""Trainium2 Bass kernel for ActivationGATSingleHeadLayer (GNN message passing).

Reference computation (jax):
    e = relu(sum(z[src] * z[dst], -1))             # [E]
    alpha = segment_softmax(e, dst)                # two-pass in ref
    h = segment_sum(alpha[:, None] * z[src], dst)  # [N, D]
    out = relu(batchnorm(h))                       # training-mode stats

Strategy (8 NeuronCores):
  * Host shards edges by dst range: core c owns dst in [c*NPC, (c+1)*NPC).
    All segment reductions are core-local; the only collective is an
    AllReduce of 128 floats of BatchNorm statistics.
  * Segment softmax is collapsed to one pass:
        h[n] = sum_e w_e * z[src_e] / sum_e w_e,  w_e = exp(relu(e_e) - SHIFT)
    The constant SHIFT (=64) replaces the segment max: relu makes e >= 0 and
    e <= max ||z_i||^2 ~ chi2_64 stays far below SHIFT + 88, so exp never
    overflows and the result is mathematically identical.
  * Edges are sorted by dst and grouped into 128-node windows. Per 128-edge
    tile, one-hot membership matrices are built arithmetically:
        onehotT[n, e] = relu(1 - (dstrel_e - n)^2)     (Scalar engine, 2 ops)
        onehot[e, n]  = (iota[n] == dstrel_e)          (Vector engine, 1 op)
  * z[dst] rows are NOT gathered: they are expanded on the TensorEngine as
    psum_zd = onehotT^T @ zwin (f32, an exact row selection). This halves the
    SWDGE descriptor-generation load, which is the dominant cost (the Q7
    ucode spends ~8ns per gather index).
  * Aggregation also runs on the TensorEngine (HBM scatter-add races on
    duplicate indices): psum[win] += onehot^T @ [w*z_src | w] in bf16
    (values are linearly averaged, so bf16 rounding stays ~1e-3).
  * z[src] rows are fetched with SWDGE dma_gather (f32, 256B elements);
    int16 indices only reach 32767, so tiles are segregated into lo
    (src < SPLIT) / hi sections gathering from the two z-table halves.
    SWDGE ops are capped at 1024 indices (larger ops overflow the
    descriptor-ring carveout and hang the device).
  * h is stored feature-major [128, D, NW] so BatchNorm stats reduce over
    contiguous memory; stats cross partitions via one matmul against ones,
    AllReduce of 128 floats, partition-broadcast back, normalize + relu.
"""

import sys

for _p in ("/opt/trn_rl_repo", "/root/.axon_site/_ro/trn_rl_repo"):
    if _p not in sys.path:
        sys.path.append(_p)

import ml_dtypes
import numpy as np

# ---------------------------------------------------------------- geometry
N_NODES = 50000
N_EDGES = 800000
D = 64
NCORES = 8

EPS = 1e-5          # BatchNorm eps (matches reference)
TINY = 1e-30        # denom guard for isolated nodes
SHIFT = 64.0        # constant subtracted inside exp
WIN = 128           # nodes per aggregation window (= PSUM partition dim)
MAX_PSUM_WIN = 5    # windows per phase: 1 bank each + 2 banks zdst + 1 stats
CHUNK_TILES = 8     # tiles per gather chunk; SWDGE ops above ~1024 indices
                    # overflow the descriptor-ring carveout and hang


def _derive(n_nodes, split):
    npc = n_nodes // NCORES
    nw = -(-npc // WIN)
    return dict(
        n_nodes=n_nodes,
        npc=npc,
        nw=nw,
        h_rows=nw * WIN,
        split=split,
    )


CFG = _derive(N_NODES, split=25000)


# ---------------------------------------------------------------- host prep
def _wrap_tile_idx(arr):
    """[T, 128] int -> [128, T, 8] int16 SWDGE layout, partition-major:
    within-tile edge p at [p%16 (+16g), tile, p//16]."""
    t = arr.shape[0]
    w = arr.reshape(t, 8, 16).transpose(0, 2, 1).astype(np.int16)  # [T,16,8]
    w = np.tile(w, (1, 8, 1))                                      # [T,128,8]
    return w.transpose(1, 0, 2).copy()                             # [128,T,8]


def prep_inputs(z, src, dst, gamma, beta, cfg=CFG):
    """Shard edges by dst range, sort by dst, build window/tile plan.

    Returns (in_maps, plan). The plan (tile metadata, section table kinds,
    chunking) is identical across cores, as SPMD requires.
    """
    z = np.ascontiguousarray(np.asarray(z, dtype=np.float32))
    src = np.asarray(src).astype(np.int64)
    dst = np.asarray(dst).astype(np.int64)
    gamma = np.asarray(gamma, dtype=np.float32)
    beta = np.asarray(beta, dtype=np.float32)

    npc, split, nw = cfg["npc"], cfg["split"], cfg["nw"]
    h_rows = cfg["h_rows"]

    # per-core, per-window, per-type edge lists
    edges = [[[None, None] for _ in range(nw)] for _ in range(NCORES)]
    core_of = dst // npc
    for c in range(NCORES):
        m = core_of == c
        s, ld = src[m], dst[m] - c * npc
        order = np.argsort(ld, kind="stable")
        s, ld = s[order], ld[order]
        w_of = ld // WIN
        lo = s < split
        for w in range(nw):
            wm = w_of == w
            for ty, sel in ((0, wm & lo), (1, wm & ~lo)):
                sg, dg = s[sel], ld[sel]
                o2 = np.argsort(sg, kind="stable")
                off = split if ty else 0
                edges[c][w][ty] = (sg[o2] - off, dg[o2])

    # equalized tile counts (identical across cores)
    nt = np.zeros((2, nw), dtype=np.int64)
    for ty in range(2):
        for w in range(nw):
            mx = max(len(edges[c][w][ty][0]) for c in range(NCORES))
            nt[ty, w] = -(-mx // 128)
    nt[0] = np.maximum(nt[0], 1)  # every window needs >= 1 tile (PSUM init)

    # phases / sections / global tile order
    phases = [list(range(i, min(i + MAX_PSUM_WIN, nw)))
              for i in range(0, nw, MAX_PSUM_WIN)]
    sections = []   # (ty, [(w, local_tile_j), ...]) in global tile order
    tile_meta = []  # (window, start, stop) per global tile
    for ph in phases:
        for ty in range(2):
            tl = []
            for w in ph:
                for j in range(nt[ty, w]):
                    start = ty == 0 and j == 0
                    stop = (
                        (ty == 1 and j == nt[1, w] - 1)
                        if nt[1, w] > 0
                        else (ty == 0 and j == nt[0, w] - 1)
                    )
                    tile_meta.append((w, start, stop))
                    tl.append((w, j))
            if tl:
                sections.append((ty, tl))

    t_total = len(tile_meta)
    plan = dict(
        cfg=cfg,
        nt=nt,
        phases=phases,
        sections=sections,
        tile_meta=tile_meta,
        t_total=t_total,
    )

    gb = np.stack([gamma, beta]).astype(np.float32)
    iota = np.arange(128, dtype=np.float32)
    niota = -iota

    in_maps = []
    for c in range(NCORES):
        isrc = np.zeros((t_total, 128), dtype=np.int64)
        drel = np.full((t_total, 128), -1.0, dtype=np.float32)
        g = 0
        for ty, tl in sections:
            for w, j in tl:
                s, ld = edges[c][w][ty]
                seg_s = s[j * 128 : (j + 1) * 128]
                seg_d = ld[j * 128 : (j + 1) * 128]
                k = len(seg_s)
                isrc[g, :k] = seg_s
                drel[g, :k] = (seg_d - w * WIN).astype(np.float32)
                g += 1
        assert g == t_total

        zs = np.zeros((h_rows, D), dtype=np.float32)
        zs[:npc] = z[c * npc : (c + 1) * npc]

        in_maps.append(
            {
                "z": z,
                "zs": zs,
                "isrc": _wrap_tile_idx(isrc),
                "drel": drel.T.copy(),          # [128, T] edge-partition-major
                "drelf": drel.reshape(-1).astype(ml_dtypes.bfloat16),
                "gb": gb,
                "iota": iota,
                "niota": niota,
            }
        )
    return in_maps, plan


# ---------------------------------------------------------------- device graph
def build_nc(plan, n_total_nodes=None):
    """Build the SPMD Bass graph (identical on all cores)."""
    from concourse import bacc, tile
    from concourse.bass import mybir

    f32 = mybir.dt.float32
    bf16 = mybir.dt.bfloat16
    i16 = mybir.dt.int16
    AX = mybir.AxisListType
    ALU = mybir.AluOpType
    ACTF = mybir.ActivationFunctionType

    cfg = plan["cfg"]
    nw, split = cfg["nw"], cfg["split"]
    h_rows, n_nodes = cfg["h_rows"], cfg["n_nodes"]
    if n_total_nodes is None:
        n_total_nodes = n_nodes
    t_total = plan["t_total"]
    tile_meta = plan["tile_meta"]

    nc = bacc.Bacc(
        "TRN2",
        target_bir_lowering=False,
        debug=False,
        num_devices=NCORES,
        num_swdge_queues=2,
    )

    z_d = nc.dram_tensor("z", [n_nodes, D], f32, kind="ExternalInput")
    zs_d = nc.dram_tensor("zs", [h_rows, D], f32, kind="ExternalInput")
    isrc_d = nc.dram_tensor("isrc", [128, t_total, 8], i16, kind="ExternalInput")
    drel_d = nc.dram_tensor("drel", [128, t_total], f32, kind="ExternalInput")
    drelf_d = nc.dram_tensor("drelf", [t_total * 128], bf16, kind="ExternalInput")
    gb_d = nc.dram_tensor("gb", [2, D], f32, kind="ExternalInput")
    iota_d = nc.dram_tensor("iota", [128], f32, kind="ExternalInput")
    niota_d = nc.dram_tensor("niota", [128], f32, kind="ExternalInput")
    out_d = nc.dram_tensor("out", [h_rows, D], f32, kind="ExternalOutput")

    CT = CHUNK_TILES

    with tile.TileContext(nc) as tc:
        with (
            tc.tile_pool(name="const", bufs=1) as constp,
            tc.tile_pool(name="data", bufs=7) as datap,
            tc.tile_pool(name="oh", bufs=6) as ohp,
            tc.tile_pool(name="small", bufs=6) as smallp,
            tc.tile_pool(name="fin", bufs=1) as finp,
            tc.tile_pool(name="dram", bufs=1, space="DRAM") as dramp,
            tc.tile_pool(name="psum", bufs=1, space="PSUM") as psump,
            tc.tile_pool(name="psz", bufs=3, space="PSUM") as pszp,
        ):
            shiftb = constp.tile([128, 1], f32)
            nc.vector.memset(shiftb[:], -SHIFT)
            epsb = constp.tile([128, 1], f32)
            nc.vector.memset(epsb[:], EPS)
            ones = constp.tile([128, 1], f32)
            nc.vector.memset(ones[:], 1.0)
            iotat = constp.tile([128, 128], f32)
            nc.sync.dma_start(iotat[:], iota_d.ap().partition_broadcast(128))
            niotac = constp.tile([128, 1], f32)
            nc.sync.dma_start(niotac[:], niota_d.ap().unsqueeze(1))

            # preload all SWDGE indices + dstrel (partition-major layouts)
            isrc_sb = constp.tile([128, t_total, 8], i16)
            nc.sync.dma_start(isrc_sb[:], isrc_d[:, :, :])
            drel_sb = constp.tile([128, t_total], f32)
            nc.sync.dma_start(drel_sb[:], drel_d[:, :])

            h_all = finp.tile([128, D, nw], f32)

            g = 0  # global tile cursor
            kq = 0  # chunk counter for SWDGE queue alternation
            for ph in plan["phases"]:
                nwp = len(ph)
                w0 = ph[0]
                psb = psump.tile([128, MAX_PSUM_WIN, 512], f32, tag="psb")
                zwin = datap.tile([128, MAX_PSUM_WIN, D], f32, tag="zwin")
                nc.sync.dma_start(
                    zwin[:, 0:nwp, :],
                    zs_d[w0 * WIN : (w0 + nwp) * WIN, :].rearrange(
                        "(w p) d -> p w d", p=128
                    ),
                )
                ph_secs = [
                    (ty, tl) for (ty, tl) in plan["sections"] if tl[0][0] in ph
                ]
                for ty, tl in ph_secs:
                    table = z_d[0:split, :] if ty == 0 else z_d[split:n_nodes, :]
                    for c0 in range(0, len(tl), CT):
                        ct = min(CT, len(tl) - c0)
                        t0 = g + c0
                        ne = ct * 128

                        zsrc = datap.tile([128, CT, D], f32, tag="zsrc")
                        nc.gpsimd.dma_gather(
                            zsrc[:, 0:ct, :],
                            table,
                            isrc_sb[:, t0 : t0 + ct, :],
                            ne,
                            ne,
                            D,
                            queue_num=kq % 2,
                        )
                        kq += 1

                        # transposed one-hot on the Scalar engine:
                        # ohT[n, e] = relu(1 - (drel_e - n)^2)
                        drbc = ohp.tile([128, CT * 128], bf16, tag="drbc")
                        nc.sync.dma_start(
                            drbc[:, 0:ne],
                            drelf_d[t0 * 128 : t0 * 128 + ne].partition_broadcast(
                                128
                            ),
                        )
                        ohT = ohp.tile([128, CT * 128], f32, tag="ohT")
                        nc.scalar.activation(
                            ohT[:, 0:ne], drbc[:, 0:ne], ACTF.Square,
                            bias=niotac[:], scale=1.0,
                        )
                        nc.scalar.activation(
                            ohT[:, 0:ne], ohT[:, 0:ne], ACTF.Relu,
                            bias=ones[:], scale=-1.0,
                        )

                        # z[dst] expansion: psum_zd[:, t, :] = ohT_t^T @ zwin_t
                        pzd = pszp.tile([128, CT, D], f32, tag="zd")
                        for tl_i in range(ct):
                            win = tile_meta[t0 + tl_i][0]
                            nc.tensor.matmul(
                                pzd[:, tl_i, :],
                                ohT[:, tl_i * 128 : (tl_i + 1) * 128],
                                zwin[:, win - w0, :],
                                start=tl_i == 0,
                                stop=tl_i == ct - 1,
                            )

                        # edge scores and weights
                        prod = datap.tile([128, CT, D], f32, tag="prod")
                        e = smallp.tile([128, CT], f32, tag="e")
                        wt = smallp.tile([128, CT], f32, tag="wt")
                        nc.vector.tensor_mul(
                            prod[:, 0:ct, :], zsrc[:, 0:ct, :], pzd[:, 0:ct, :]
                        )
                        nc.vector.tensor_reduce(
                            e[:, 0:ct], prod[:, 0:ct, :], axis=AX.X, op=ALU.add
                        )
                        nc.vector.tensor_scalar_max(e[:, 0:ct], e[:, 0:ct], 0.0)
                        nc.scalar.activation(
                            wt[:, 0:ct], e[:, 0:ct], ACTF.Exp,
                            bias=shiftb[:], scale=1.0,
                        )

                        # vals = [w * z_src | w] in bf16
                        vals = datap.tile([128, CT, D + 1], bf16, tag="vals")
                        nc.scalar.copy(vals[:, 0:ct, D], wt[:, 0:ct])
                        nc.vector.tensor_mul(
                            vals[:, 0:ct, 0:D],
                            zsrc[:, 0:ct, :],
                            wt[:, 0:ct].unsqueeze(2).broadcast_to((128, ct, D)),
                        )

                        # aggregation one-hot (edge-major) in bf16
                        oh = ohp.tile([128, CT, 128], bf16, tag="oh")
                        nc.vector.tensor_tensor(
                            oh[:, 0:ct, :],
                            iotat[:].unsqueeze(1).broadcast_to((128, ct, 128)),
                            drel_sb[:, t0 : t0 + ct]
                            .unsqueeze(2)
                            .broadcast_to((128, ct, 128)),
                            op=ALU.is_equal,
                        )

                        for tl_i in range(ct):
                            win, st, sp = tile_meta[t0 + tl_i]
                            slot = win - w0
                            nc.tensor.matmul(
                                psb[:, slot, 0 : D + 1],
                                oh[:, tl_i, :],
                                vals[:, tl_i, :],
                                start=st,
                                stop=sp,
                            )
                    g += len(tl)

                # drain phase: h = num / denom, written feature-major
                denp = smallp.tile([128, MAX_PSUM_WIN], f32, tag="den")
                recp = smallp.tile([128, MAX_PSUM_WIN], f32, tag="rec")
                nc.vector.tensor_scalar_add(
                    denp[:, 0:nwp], psb[:, 0:nwp, D], TINY
                )
                nc.vector.reciprocal(recp[:, 0:nwp], denp[:, 0:nwp])
                nc.vector.tensor_mul(
                    h_all[:, :, w0 : w0 + nwp].transpose((0, 2, 1)),
                    psb[:, 0:nwp, 0:D],
                    recp[:, 0:nwp].unsqueeze(2).broadcast_to((128, nwp, D)),
                )

            # ---- BatchNorm stats: s1 = sum(h), s2 = sum(h^2) over all nodes
            hsq = finp.tile([128, D, nw], f32, tag="hsq")
            nc.scalar.square(hsq[:], h_all[:])

            stats = smallp.tile([128, 2 * D], f32, tag="stats")
            nc.vector.tensor_reduce(
                stats[:, 0:D], h_all[:], axis=AX.X, op=ALU.add
            )
            nc.vector.tensor_reduce(
                stats[:, D : 2 * D], hsq[:], axis=AX.X, op=ALU.add
            )

            ps = pszp.tile([1, 2 * D], f32, tag="zd")
            nc.tensor.matmul(ps[:], ones[:], stats[:], start=True, stop=True)
            srow = smallp.tile([1, 2 * D], f32, tag="srow")
            nc.scalar.copy(srow[:], ps[:])

            cc_in = dramp.tile([1, 2 * D], f32)
            cc_out = dramp.tile([1, 2 * D], f32)
            nc.sync.dma_start(cc_in[:], srow[:])
            nc.gpsimd.collective_compute(
                "AllReduce",
                ALU.add,
                ins=[cc_in.opt()],
                outs=[cc_out.opt()],
                replica_groups=[list(range(NCORES))],
            )

            G = smallp.tile([128, 2 * D], f32, tag="G")
            nc.sync.dma_start(G[:], cc_out[:].squeeze(0).partition_broadcast(128))
            gbB = constp.tile([128, 2 * D], f32)
            nc.sync.dma_start(gbB[:], gb_d.ap().flatten().partition_broadcast(128))

            inv_n = 1.0 / float(n_total_nodes)
            mean = smallp.tile([128, D], f32, tag="mean")
            var = smallp.tile([128, D], f32, tag="var")
            nc.scalar.mul(mean[:], G[:, 0:D], inv_n)
            nc.scalar.mul(var[:], G[:, D : 2 * D], inv_n)
            msq = smallp.tile([128, D], f32, tag="msq")
            nc.vector.tensor_mul(msq[:], mean[:], mean[:])
            nc.vector.tensor_sub(var[:], var[:], msq[:])
            std = smallp.tile([128, D], f32, tag="std")
            nc.scalar.activation(std[:], var[:], ACTF.Sqrt, bias=epsb[:], scale=1.0)
            rstd = smallp.tile([128, D], f32, tag="rstd")
            nc.vector.reciprocal(rstd[:], std[:])

            a = smallp.tile([128, D], f32, tag="a")
            b = smallp.tile([128, D], f32, tag="b")
            nc.vector.tensor_mul(a[:], gbB[:, 0:D], rstd[:])
            nc.vector.tensor_mul(b[:], mean[:], a[:])
            nc.vector.tensor_sub(b[:], gbB[:, D : 2 * D], b[:])

            # y stored node-major so the output DMA gets 256B-contiguous runs
            y = finp.tile([128, nw, D], f32, tag="hsq")
            nc.vector.tensor_mul(
                y[:],
                h_all[:].transpose((0, 2, 1)),
                a[:].unsqueeze(1).broadcast_to((128, nw, D)),
            )
            nc.vector.tensor_add(
                y[:], y[:], b[:].unsqueeze(1).broadcast_to((128, nw, D))
            )
            nc.vector.tensor_relu(y[:], y[:])

            outv = out_d.ap().rearrange("(c p) f -> p c f", p=128)
            nc.sync.dma_start(outv, y[:])

    nc.compile()
    return nc


# ---------------------------------------------------------------- entry point
TRACE = False          # set True by test harnesses to capture exec_time_ns
LAST_RESULT = None     # BassKernelResults of the most recent kernel() call


def kernel(**inputs):
    z = inputs["z"]
    src = inputs["src"]
    dst = inputs["dst"]
    gamma = inputs["gamma"]
    beta = inputs["beta"]

    from concourse.bass_utils import run_bass_kernel_spmd

    in_maps, plan = prep_inputs(z, src, dst, gamma, beta)
    nc = build_nc(plan)
    res = run_bass_kernel_spmd(
        nc, in_maps, core_ids=list(range(NCORES)), trace=TRACE
    )
    global LAST_RESULT
    LAST_RESULT = res

    npc = CFG["npc"]
    out = np.empty((N_NODES, D), dtype=np.float32)
    for c in range(NCORES):
        out[c * npc : (c + 1) * npc] = res.results[c]["out"][:npc]
    return out


# revision 37
# speedup vs baseline: 1.2838x; 1.0009x over previous
"""Trainium2 Bass kernel for ActivationGATSingleHeadLayer (GNN message passing).

Reference computation (jax):
    e = relu(sum(z[src] * z[dst], -1))             # [E]
    alpha = segment_softmax(e, dst)                # two-pass in ref
    h = segment_sum(alpha[:, None] * z[src], dst)  # [N, D]
    out = relu(batchnorm(h))                       # training-mode stats

Strategy (8 NeuronCores):
  * Host shards edges by dst range: core c owns dst in [c*NPC, (c+1)*NPC).
    All segment reductions are core-local; the only collective is an
    AllReduce of 128 floats of BatchNorm statistics.
  * Segment softmax is collapsed to one pass:
        h[n] = sum_e w_e * z[src_e] / sum_e w_e,  w_e = exp(relu(e_e) - SHIFT)
    The constant SHIFT (=64) replaces the segment max: relu makes e >= 0 and
    e <= max ||z_i||^2 ~ chi2_64 stays far below SHIFT + 88, so exp never
    overflows and the result is mathematically identical.
  * Edges are sorted by dst and grouped into 128-node windows. Per 128-edge
    tile, one-hot membership matrices are built arithmetically:
        onehotT[n, e] = relu(1 - (dstrel_e - n)^2)     (Scalar engine, 2 ops)
        onehot[e, n]  = (iota[n] == dstrel_e)          (Vector engine, 1 op)
  * z[dst] rows are NOT gathered: they are expanded on the TensorEngine as
    psum_zd = onehotT^T @ zwin (f32, an exact row selection). This halves the
    SWDGE descriptor-generation load, which is the dominant cost (the Q7
    ucode spends ~8ns per gather index).
  * Aggregation also runs on the TensorEngine (HBM scatter-add races on
    duplicate indices): psum[win] += onehot^T @ [w*z_src | w] in bf16
    (values are linearly averaged, so bf16 rounding stays ~1e-3).
  * z[src] rows are fetched with SWDGE dma_gather (f32, 256B elements);
    int16 indices only reach 32767, so tiles are segregated into lo
    (src < SPLIT) / hi sections gathering from the two z-table halves.
    SWDGE ops are capped at 1024 indices (larger ops overflow the
    descriptor-ring carveout and hang the device).
  * h is stored feature-major [128, D, NW] so BatchNorm stats reduce over
    contiguous memory; stats cross partitions via one matmul against ones,
    AllReduce of 128 floats, partition-broadcast back, normalize + relu.
"""

import sys

for _p in ("/opt/trn_rl_repo", "/root/.axon_site/_ro/trn_rl_repo"):
    if _p not in sys.path:
        sys.path.append(_p)

import ml_dtypes
import numpy as np

# ---------------------------------------------------------------- geometry
N_NODES = 50000
N_EDGES = 800000
D = 64
NCORES = 8

EPS = 1e-5          # BatchNorm eps (matches reference)
TINY = 1e-30        # denom guard for isolated nodes
SHIFT = 64.0        # constant subtracted inside exp
WIN = 128           # nodes per aggregation window (= PSUM partition dim)
MAX_PSUM_WIN = 5    # windows per phase: 1 bank each + 2 banks zdst + 1 stats
CHUNK_TILES = 8     # tiles per gather chunk; SWDGE ops above ~1024 indices
                    # overflow the descriptor-ring carveout and hang


def _derive(n_nodes, split):
    npc = n_nodes // NCORES
    nw = -(-npc // WIN)
    return dict(
        n_nodes=n_nodes,
        npc=npc,
        nw=nw,
        h_rows=nw * WIN,
        split=split,
    )


CFG = _derive(N_NODES, split=25000)


# ---------------------------------------------------------------- host prep
def _wrap_tile_idx(arr):
    """[T, 128] int -> [128, T, 8] int16 SWDGE layout, partition-major:
    within-tile edge p at [p%16 (+16g), tile, p//16]."""
    t = arr.shape[0]
    w = arr.reshape(t, 8, 16).transpose(0, 2, 1).astype(np.int16)  # [T,16,8]
    w = np.tile(w, (1, 8, 1))                                      # [T,128,8]
    return w.transpose(1, 0, 2).copy()                             # [128,T,8]


def prep_inputs(z, src, dst, gamma, beta, cfg=CFG):
    """Shard edges by dst range, sort by dst, build window/tile plan.

    Returns (in_maps, plan). The plan (tile metadata, section table kinds,
    chunking) is identical across cores, as SPMD requires.
    """
    z = np.ascontiguousarray(np.asarray(z, dtype=np.float32))
    src = np.asarray(src).astype(np.int64)
    dst = np.asarray(dst).astype(np.int64)
    gamma = np.asarray(gamma, dtype=np.float32)
    beta = np.asarray(beta, dtype=np.float32)

    npc, split, nw = cfg["npc"], cfg["split"], cfg["nw"]
    h_rows = cfg["h_rows"]

    # per-core, per-window, per-type edge lists
    edges = [[[None, None] for _ in range(nw)] for _ in range(NCORES)]
    core_of = dst // npc
    for c in range(NCORES):
        m = core_of == c
        s, ld = src[m], dst[m] - c * npc
        order = np.argsort(ld, kind="stable")
        s, ld = s[order], ld[order]
        w_of = ld // WIN
        lo = s < split
        for w in range(nw):
            wm = w_of == w
            for ty, sel in ((0, wm & lo), (1, wm & ~lo)):
                sg, dg = s[sel], ld[sel]
                o2 = np.argsort(sg, kind="stable")
                off = split if ty else 0
                edges[c][w][ty] = (sg[o2] - off, dg[o2])

    # equalized tile counts (identical across cores)
    nt = np.zeros((2, nw), dtype=np.int64)
    for ty in range(2):
        for w in range(nw):
            mx = max(len(edges[c][w][ty][0]) for c in range(NCORES))
            nt[ty, w] = -(-mx // 128)
    nt[0] = np.maximum(nt[0], 1)  # every window needs >= 1 tile (PSUM init)

    # phases / sections / global tile order
    phases = [list(range(i, min(i + MAX_PSUM_WIN, nw)))
              for i in range(0, nw, MAX_PSUM_WIN)]
    sections = []   # (ty, [(w, local_tile_j), ...]) in global tile order
    tile_meta = []  # (window, start, stop) per global tile
    for ph in phases:
        for ty in range(2):
            tl = []
            for w in ph:
                for j in range(nt[ty, w]):
                    start = ty == 0 and j == 0
                    stop = (
                        (ty == 1 and j == nt[1, w] - 1)
                        if nt[1, w] > 0
                        else (ty == 0 and j == nt[0, w] - 1)
                    )
                    tile_meta.append((w, start, stop))
                    tl.append((w, j))
            if tl:
                sections.append((ty, tl))

    t_total = len(tile_meta)
    plan = dict(
        cfg=cfg,
        nt=nt,
        phases=phases,
        sections=sections,
        tile_meta=tile_meta,
        t_total=t_total,
    )

    gb = np.stack([gamma, beta]).astype(np.float32)
    iota = np.arange(128, dtype=np.float32)
    niota = -iota

    in_maps = []
    for c in range(NCORES):
        isrc = np.zeros((t_total, 128), dtype=np.int64)
        drel = np.full((t_total, 128), -1.0, dtype=np.float32)
        g = 0
        for ty, tl in sections:
            for w, j in tl:
                s, ld = edges[c][w][ty]
                seg_s = s[j * 128 : (j + 1) * 128]
                seg_d = ld[j * 128 : (j + 1) * 128]
                k = len(seg_s)
                isrc[g, :k] = seg_s
                drel[g, :k] = (seg_d - w * WIN).astype(np.float32)
                g += 1
        assert g == t_total

        zs = np.zeros((h_rows, D), dtype=np.float32)
        zs[:npc] = z[c * npc : (c + 1) * npc]

        in_maps.append(
            {
                "z": z,
                "zs": zs,
                "isrc": _wrap_tile_idx(isrc),
                "drel": drel.T.astype(ml_dtypes.bfloat16),  # [128, T] edge-major
                "drelf": drel.reshape(-1).astype(ml_dtypes.bfloat16),
                "gb": gb,
                "iota": iota,
                "niota": niota,
            }
        )
    return in_maps, plan


# ---------------------------------------------------------------- device graph
def build_nc(plan, n_total_nodes=None):
    """Build the SPMD Bass graph (identical on all cores)."""
    from concourse import bacc, tile
    from concourse.bass import mybir

    f32 = mybir.dt.float32
    bf16 = mybir.dt.bfloat16
    i16 = mybir.dt.int16
    AX = mybir.AxisListType
    ALU = mybir.AluOpType
    ACTF = mybir.ActivationFunctionType

    cfg = plan["cfg"]
    nw, split = cfg["nw"], cfg["split"]
    h_rows, n_nodes = cfg["h_rows"], cfg["n_nodes"]
    if n_total_nodes is None:
        n_total_nodes = n_nodes
    t_total = plan["t_total"]
    tile_meta = plan["tile_meta"]

    nc = bacc.Bacc(
        "TRN2",
        target_bir_lowering=False,
        debug=False,
        num_devices=NCORES,
        num_swdge_queues=4,
    )

    z_d = nc.dram_tensor("z", [n_nodes, D], f32, kind="ExternalInput")
    zs_d = nc.dram_tensor("zs", [h_rows, D], f32, kind="ExternalInput")
    isrc_d = nc.dram_tensor("isrc", [128, t_total, 8], i16, kind="ExternalInput")
    drel_d = nc.dram_tensor("drel", [128, t_total], bf16, kind="ExternalInput")
    drelf_d = nc.dram_tensor("drelf", [t_total * 128], bf16, kind="ExternalInput")
    gb_d = nc.dram_tensor("gb", [2, D], f32, kind="ExternalInput")
    iota_d = nc.dram_tensor("iota", [128], f32, kind="ExternalInput")
    niota_d = nc.dram_tensor("niota", [128], f32, kind="ExternalInput")
    out_d = nc.dram_tensor("out", [h_rows, D], f32, kind="ExternalOutput")

    CT = CHUNK_TILES

    with tile.TileContext(nc) as tc:
        with (
            tc.tile_pool(name="const", bufs=1) as constp,
            tc.tile_pool(name="data", bufs=7) as datap,
            tc.tile_pool(name="oh", bufs=6) as ohp,
            tc.tile_pool(name="small", bufs=6) as smallp,
            tc.tile_pool(name="fin", bufs=1) as finp,
            tc.tile_pool(name="dram", bufs=1, space="DRAM") as dramp,
            tc.tile_pool(name="psum", bufs=1, space="PSUM") as psump,
            tc.tile_pool(name="psz", bufs=3, space="PSUM") as pszp,
        ):
            shiftb = constp.tile([128, 1], f32)
            nc.vector.memset(shiftb[:], -SHIFT)
            epsb = constp.tile([128, 1], f32)
            nc.vector.memset(epsb[:], EPS)
            ones = constp.tile([128, 1], f32)
            nc.vector.memset(ones[:], 1.0)
            iotat = constp.tile([128, 128], bf16)
            iotf32 = constp.tile([128, 128], f32)
            nc.sync.dma_start(iotf32[:], iota_d.ap().partition_broadcast(128))
            nc.vector.tensor_copy(iotat[:], iotf32[:])
            niotac = constp.tile([128, 1], f32)
            nc.sync.dma_start(niotac[:], niota_d.ap().unsqueeze(1))

            # preload all SWDGE indices + dstrel (partition-major layouts)
            isrc_sb = constp.tile([128, t_total, 8], i16)
            nc.sync.dma_start(isrc_sb[:], isrc_d[:, :, :])
            drel_sb = constp.tile([128, t_total], bf16)
            nc.sync.dma_start(drel_sb[:], drel_d[:, :])

            h_all = finp.tile([128, D, nw], f32)

            g = 0  # global tile cursor
            kq = 0  # chunk counter for SWDGE queue alternation
            for ph in plan["phases"]:
                nwp = len(ph)
                w0 = ph[0]
                psb = psump.tile([128, MAX_PSUM_WIN, 512], f32, tag="psb")
                zwin = datap.tile([128, MAX_PSUM_WIN, D], f32, tag="zwin")
                nc.sync.dma_start(
                    zwin[:, 0:nwp, :],
                    zs_d[w0 * WIN : (w0 + nwp) * WIN, :].rearrange(
                        "(w p) d -> p w d", p=128
                    ),
                )
                ph_secs = [
                    (ty, tl) for (ty, tl) in plan["sections"] if tl[0][0] in ph
                ]
                for ty, tl in ph_secs:
                    table = z_d[0:split, :] if ty == 0 else z_d[split:n_nodes, :]
                    for c0 in range(0, len(tl), CT):
                        ct = min(CT, len(tl) - c0)
                        t0 = g + c0
                        ne = ct * 128

                        zsrc = datap.tile([128, CT, D], f32, tag="zsrc")
                        nc.gpsimd.dma_gather(
                            zsrc[:, 0:ct, :],
                            table,
                            isrc_sb[:, t0 : t0 + ct, :],
                            ne,
                            ne,
                            D,
                            queue_num=kq % 4,
                        )
                        kq += 1

                        # transposed one-hot on the Scalar engine:
                        # ohT[n, e] = relu(1 - (drel_e - n)^2)
                        drbc = ohp.tile([128, CT * 128], bf16, tag="drbc")
                        nc.sync.dma_start(
                            drbc[:, 0:ne],
                            drelf_d[t0 * 128 : t0 * 128 + ne].partition_broadcast(
                                128
                            ),
                        )
                        ohT = ohp.tile([128, CT * 128], f32, tag="ohT")
                        nc.scalar.activation(
                            ohT[:, 0:ne], drbc[:, 0:ne], ACTF.Square,
                            bias=niotac[:], scale=1.0,
                        )
                        nc.scalar.activation(
                            ohT[:, 0:ne], ohT[:, 0:ne], ACTF.Relu,
                            bias=ones[:], scale=-1.0,
                        )

                        # z[dst] expansion: psum_zd[:, t, :] = ohT_t^T @ zwin_t
                        pzd = pszp.tile([128, CT, D], f32, tag="zd")
                        for tl_i in range(ct):
                            win = tile_meta[t0 + tl_i][0]
                            nc.tensor.matmul(
                                pzd[:, tl_i, :],
                                ohT[:, tl_i * 128 : (tl_i + 1) * 128],
                                zwin[:, win - w0, :],
                                start=tl_i == 0,
                                stop=tl_i == ct - 1,
                            )

                        # edge scores and weights
                        prod = datap.tile([128, CT, D], f32, tag="prod")
                        e = smallp.tile([128, CT], f32, tag="e")
                        wt = smallp.tile([128, CT], f32, tag="wt")
                        nc.vector.tensor_mul(
                            prod[:, 0:ct, :], zsrc[:, 0:ct, :], pzd[:, 0:ct, :]
                        )
                        nc.vector.tensor_reduce(
                            e[:, 0:ct], prod[:, 0:ct, :], axis=AX.X, op=ALU.add
                        )
                        nc.vector.tensor_scalar_max(e[:, 0:ct], e[:, 0:ct], 0.0)
                        nc.scalar.activation(
                            wt[:, 0:ct], e[:, 0:ct], ACTF.Exp,
                            bias=shiftb[:], scale=1.0,
                        )

                        # vals = [w * z_src | w] in bf16
                        vals = datap.tile([128, CT, D + 1], bf16, tag="vals")
                        nc.scalar.copy(vals[:, 0:ct, D], wt[:, 0:ct])
                        nc.vector.tensor_mul(
                            vals[:, 0:ct, 0:D],
                            zsrc[:, 0:ct, :],
                            wt[:, 0:ct].unsqueeze(2).broadcast_to((128, ct, D)),
                        )

                        # aggregation one-hot (edge-major) in bf16
                        oh = ohp.tile([128, CT, 128], bf16, tag="oh")
                        nc.vector.tensor_tensor(
                            oh[:, 0:ct, :],
                            iotat[:].unsqueeze(1).broadcast_to((128, ct, 128)),
                            drel_sb[:, t0 : t0 + ct]
                            .unsqueeze(2)
                            .broadcast_to((128, ct, 128)),
                            op=ALU.is_equal,
                        )

                        for tl_i in range(ct):
                            win, st, sp = tile_meta[t0 + tl_i]
                            slot = win - w0
                            nc.tensor.matmul(
                                psb[:, slot, 0 : D + 1],
                                oh[:, tl_i, :],
                                vals[:, tl_i, :],
                                start=st,
                                stop=sp,
                            )
                    g += len(tl)

                # drain phase: h = num / denom, written feature-major
                denp = smallp.tile([128, MAX_PSUM_WIN], f32, tag="den")
                recp = smallp.tile([128, MAX_PSUM_WIN], f32, tag="rec")
                nc.vector.tensor_scalar_add(
                    denp[:, 0:nwp], psb[:, 0:nwp, D], TINY
                )
                nc.vector.reciprocal(recp[:, 0:nwp], denp[:, 0:nwp])
                nc.vector.tensor_mul(
                    h_all[:, :, w0 : w0 + nwp].transpose((0, 2, 1)),
                    psb[:, 0:nwp, 0:D],
                    recp[:, 0:nwp].unsqueeze(2).broadcast_to((128, nwp, D)),
                )

            # ---- BatchNorm stats: s1 = sum(h), s2 = sum(h^2) over all nodes
            hsq = finp.tile([128, D, nw], f32, tag="hsq")
            nc.scalar.square(hsq[:], h_all[:])

            stats = smallp.tile([128, 2 * D], f32, tag="stats")
            nc.vector.tensor_reduce(
                stats[:, 0:D], h_all[:], axis=AX.X, op=ALU.add
            )
            nc.vector.tensor_reduce(
                stats[:, D : 2 * D], hsq[:], axis=AX.X, op=ALU.add
            )

            ps = pszp.tile([1, 2 * D], f32, tag="zd")
            nc.tensor.matmul(ps[:], ones[:], stats[:], start=True, stop=True)
            srow = smallp.tile([1, 2 * D], f32, tag="srow")
            nc.scalar.copy(srow[:], ps[:])

            cc_in = dramp.tile([1, 2 * D], f32)
            cc_out = dramp.tile([1, 2 * D], f32)
            nc.sync.dma_start(cc_in[:], srow[:])
            nc.gpsimd.collective_compute(
                "AllReduce",
                ALU.add,
                ins=[cc_in.opt()],
                outs=[cc_out.opt()],
                replica_groups=[list(range(NCORES))],
            )

            G = smallp.tile([128, 2 * D], f32, tag="G")
            nc.sync.dma_start(G[:], cc_out[:].squeeze(0).partition_broadcast(128))
            gbB = constp.tile([128, 2 * D], f32)
            nc.sync.dma_start(gbB[:], gb_d.ap().flatten().partition_broadcast(128))

            inv_n = 1.0 / float(n_total_nodes)
            mean = smallp.tile([128, D], f32, tag="mean")
            var = smallp.tile([128, D], f32, tag="var")
            nc.scalar.mul(mean[:], G[:, 0:D], inv_n)
            nc.scalar.mul(var[:], G[:, D : 2 * D], inv_n)
            msq = smallp.tile([128, D], f32, tag="msq")
            nc.vector.tensor_mul(msq[:], mean[:], mean[:])
            nc.vector.tensor_sub(var[:], var[:], msq[:])
            std = smallp.tile([128, D], f32, tag="std")
            nc.scalar.activation(std[:], var[:], ACTF.Sqrt, bias=epsb[:], scale=1.0)
            rstd = smallp.tile([128, D], f32, tag="rstd")
            nc.vector.reciprocal(rstd[:], std[:])

            a = smallp.tile([128, D], f32, tag="a")
            b = smallp.tile([128, D], f32, tag="b")
            nc.vector.tensor_mul(a[:], gbB[:, 0:D], rstd[:])
            nc.vector.tensor_mul(b[:], mean[:], a[:])
            nc.vector.tensor_sub(b[:], gbB[:, D : 2 * D], b[:])

            # y stored node-major so the output DMA gets 256B-contiguous runs
            y = finp.tile([128, nw, D], f32, tag="hsq")
            nc.vector.tensor_mul(
                y[:],
                h_all[:].transpose((0, 2, 1)),
                a[:].unsqueeze(1).broadcast_to((128, nw, D)),
            )
            nc.vector.tensor_add(
                y[:], y[:], b[:].unsqueeze(1).broadcast_to((128, nw, D))
            )
            nc.vector.tensor_relu(y[:], y[:])

            outv = out_d.ap().rearrange("(c p) f -> p c f", p=128)
            nc.sync.dma_start(outv, y[:])

    nc.compile()
    return nc


# ---------------------------------------------------------------- entry point
TRACE = False          # set True by test harnesses to capture exec_time_ns
LAST_RESULT = None     # BassKernelResults of the most recent kernel() call


def kernel(**inputs):
    z = inputs["z"]
    src = inputs["src"]
    dst = inputs["dst"]
    gamma = inputs["gamma"]
    beta = inputs["beta"]

    from concourse.bass_utils import run_bass_kernel_spmd

    in_maps, plan = prep_inputs(z, src, dst, gamma, beta)
    nc = build_nc(plan)
    res = run_bass_kernel_spmd(
        nc, in_maps, core_ids=list(range(NCORES)), trace=TRACE
    )
    global LAST_RESULT
    LAST_RESULT = res

    npc = CFG["npc"]
    out = np.empty((N_NODES, D), dtype=np.float32)
    for c in range(NCORES):
        out[c * npc : (c + 1) * npc] = res.results[c]["out"][:npc]
    return out


# revision 40
# speedup vs baseline: 1.8568x; 1.4464x over previous
"""Trainium2 Bass kernel for ActivationGATSingleHeadLayer (GNN message passing).

Reference computation (jax):
    e = relu(sum(z[src] * z[dst], -1))             # [E]
    alpha = segment_softmax(e, dst)                # two-pass in ref
    h = segment_sum(alpha[:, None] * z[src], dst)  # [N, D]
    out = relu(batchnorm(h))                       # training-mode stats

Strategy (8 NeuronCores):
  * Host shards edges by dst range: core c owns dst in [c*NPC, (c+1)*NPC).
    All segment reductions are core-local; the only collective is an
    AllReduce of 128 floats of BatchNorm statistics.
  * Segment softmax is collapsed to one pass:
        h[n] = sum_e w_e * z[src_e] / sum_e w_e,  w_e = exp(relu(e_e) - SHIFT)
    The constant SHIFT (=64) replaces the segment max: relu makes e >= 0 and
    e <= max ||z_i||^2 ~ chi2_64 stays far below SHIFT + 88, so exp never
    overflows and the result is mathematically identical.
  * Edges are sorted by dst and grouped into 128-node windows. Per 128-edge
    tile, one-hot membership matrices are built arithmetically:
        onehotT[n, e] = relu(1 - (dstrel_e - n)^2)     (Scalar engine, 2 ops)
        onehot[e, n]  = (iota[n] == dstrel_e)          (Vector engine, 1 op)
  * z[dst] rows are NOT gathered: they are expanded on the TensorEngine as
    psum_zd = onehotT^T @ zwin (f32, an exact row selection). This halves the
    SWDGE descriptor-generation load, which is the dominant cost (the Q7
    ucode spends ~8ns per gather index).
  * Aggregation also runs on the TensorEngine (HBM scatter-add races on
    duplicate indices): psum[win] += onehot^T @ [w*z_src | w] in bf16
    (values are linearly averaged, so bf16 rounding stays ~1e-3).
  * z[src] rows are fetched with SWDGE dma_gather (f32, 256B elements);
    int16 indices only reach 32767, so tiles are segregated into lo
    (src < SPLIT) / hi sections gathering from the two z-table halves.
    SWDGE ops are capped at 1024 indices (larger ops overflow the
    descriptor-ring carveout and hang the device).
  * h is stored feature-major [128, D, NW] so BatchNorm stats reduce over
    contiguous memory; stats cross partitions via one matmul against ones,
    AllReduce of 128 floats, partition-broadcast back, normalize + relu.
"""

import sys

for _p in ("/opt/trn_rl_repo", "/root/.axon_site/_ro/trn_rl_repo"):
    if _p not in sys.path:
        sys.path.append(_p)

import ml_dtypes
import numpy as np

# ---------------------------------------------------------------- geometry
N_NODES = 50000
N_EDGES = 800000
D = 64
NCORES = 8

EPS = 1e-5          # BatchNorm eps (matches reference)
TINY = 1e-30        # denom guard for isolated nodes
SHIFT = 64.0        # constant subtracted inside exp
WIN = 128           # nodes per aggregation window (= PSUM partition dim)
MAX_PSUM_WIN = 5    # windows per phase: 1 bank each + 2 banks zdst + 1 stats
CHUNK_TILES = 8     # tiles per gather chunk; SWDGE ops above ~1024 indices
                    # overflow the descriptor-ring carveout and hang


def _derive(n_nodes, split):
    npc = n_nodes // NCORES
    nw = -(-npc // WIN)
    return dict(
        n_nodes=n_nodes,
        npc=npc,
        nw=nw,
        h_rows=nw * WIN,
        split=split,
    )


CFG = _derive(N_NODES, split=25000)


# ---------------------------------------------------------------- host prep
def _wrap_tile_idx(arr):
    """[T, 128] int -> [128, T, 8] int16 SWDGE layout, partition-major:
    within-tile edge p at [p%16 (+16g), tile, p//16]."""
    t = arr.shape[0]
    w = arr.reshape(t, 8, 16).transpose(0, 2, 1).astype(np.int16)  # [T,16,8]
    w = np.tile(w, (1, 8, 1))                                      # [T,128,8]
    return w.transpose(1, 0, 2).copy()                             # [128,T,8]


def prep_inputs(z, src, dst, gamma, beta, cfg=CFG):
    """Shard edges by dst range, sort by dst, build window/tile plan.

    Returns (in_maps, plan). The plan (tile metadata, section table kinds,
    chunking) is identical across cores, as SPMD requires.
    """
    z = np.ascontiguousarray(np.asarray(z, dtype=np.float32))
    src = np.asarray(src).astype(np.int64)
    dst = np.asarray(dst).astype(np.int64)
    gamma = np.asarray(gamma, dtype=np.float32)
    beta = np.asarray(beta, dtype=np.float32)

    npc, split, nw = cfg["npc"], cfg["split"], cfg["nw"]
    h_rows = cfg["h_rows"]

    # per-core, per-window, per-type edge lists
    edges = [[[None, None] for _ in range(nw)] for _ in range(NCORES)]
    core_of = dst // npc
    for c in range(NCORES):
        m = core_of == c
        s, ld = src[m], dst[m] - c * npc
        order = np.argsort(ld, kind="stable")
        s, ld = s[order], ld[order]
        w_of = ld // WIN
        lo = s < split
        for w in range(nw):
            wm = w_of == w
            for ty, sel in ((0, wm & lo), (1, wm & ~lo)):
                sg, dg = s[sel], ld[sel]
                o2 = np.argsort(sg, kind="stable")
                off = split if ty else 0
                edges[c][w][ty] = (sg[o2] - off, dg[o2])

    # equalized tile counts (identical across cores)
    nt = np.zeros((2, nw), dtype=np.int64)
    for ty in range(2):
        for w in range(nw):
            mx = max(len(edges[c][w][ty][0]) for c in range(NCORES))
            nt[ty, w] = -(-mx // 128)
    nt[0] = np.maximum(nt[0], 1)  # every window needs >= 1 tile (PSUM init)

    # phases / sections / global tile order
    phases = [list(range(i, min(i + MAX_PSUM_WIN, nw)))
              for i in range(0, nw, MAX_PSUM_WIN)]
    sections = []   # (ty, [(w, local_tile_j), ...]) in global tile order
    tile_meta = []  # (window, start, stop) per global tile
    for ph in phases:
        for ty in range(2):
            tl = []
            for w in ph:
                for j in range(nt[ty, w]):
                    start = ty == 0 and j == 0
                    stop = (
                        (ty == 1 and j == nt[1, w] - 1)
                        if nt[1, w] > 0
                        else (ty == 0 and j == nt[0, w] - 1)
                    )
                    tile_meta.append((w, start, stop))
                    tl.append((w, j))
            if tl:
                sections.append((ty, tl))

    t_total = len(tile_meta)
    plan = dict(
        cfg=cfg,
        nt=nt,
        phases=phases,
        sections=sections,
        tile_meta=tile_meta,
        t_total=t_total,
    )

    gb = np.stack([gamma, beta]).astype(np.float32)
    iota = np.arange(128, dtype=np.float32)
    niota = -iota

    in_maps = []
    for c in range(NCORES):
        isrc = np.zeros((t_total, 128), dtype=np.int64)
        drel = np.full((t_total, 128), -1.0, dtype=np.float32)
        g = 0
        for ty, tl in sections:
            for w, j in tl:
                s, ld = edges[c][w][ty]
                seg_s = s[j * 128 : (j + 1) * 128]
                seg_d = ld[j * 128 : (j + 1) * 128]
                k = len(seg_s)
                isrc[g, :k] = seg_s
                drel[g, :k] = (seg_d - w * WIN).astype(np.float32)
                g += 1
        assert g == t_total

        zs = np.zeros((h_rows, D), dtype=np.float32)
        zs[:npc] = z[c * npc : (c + 1) * npc]

        in_maps.append(
            {
                "z": z,
                "zs": zs,
                "isrc": _wrap_tile_idx(isrc),
                "drel": drel.T.astype(ml_dtypes.bfloat16),  # [128, T] edge-major
                "drelf": drel.reshape(-1).astype(ml_dtypes.bfloat16),
                "gb": gb,
                "iota": iota,
                "niota": niota,
            }
        )
    return in_maps, plan


# ---------------------------------------------------------------- device graph
def build_nc(plan, n_total_nodes=None):
    """Build the SPMD Bass graph (identical on all cores)."""
    from concourse import bacc, tile
    from concourse.bass import mybir

    f32 = mybir.dt.float32
    bf16 = mybir.dt.bfloat16
    i16 = mybir.dt.int16
    AX = mybir.AxisListType
    ALU = mybir.AluOpType
    ACTF = mybir.ActivationFunctionType

    cfg = plan["cfg"]
    nw, split = cfg["nw"], cfg["split"]
    h_rows, n_nodes = cfg["h_rows"], cfg["n_nodes"]
    if n_total_nodes is None:
        n_total_nodes = n_nodes
    t_total = plan["t_total"]
    tile_meta = plan["tile_meta"]

    nc = bacc.Bacc(
        "TRN2",
        target_bir_lowering=False,
        debug=False,
        num_devices=NCORES,
        num_swdge_queues=4,
    )

    z_d = nc.dram_tensor("z", [n_nodes, D], f32, kind="ExternalInput")
    zs_d = nc.dram_tensor("zs", [h_rows, D], f32, kind="ExternalInput")
    isrc_d = nc.dram_tensor("isrc", [128, t_total, 8], i16, kind="ExternalInput")
    drel_d = nc.dram_tensor("drel", [128, t_total], bf16, kind="ExternalInput")
    drelf_d = nc.dram_tensor("drelf", [t_total * 128], bf16, kind="ExternalInput")
    gb_d = nc.dram_tensor("gb", [2, D], f32, kind="ExternalInput")
    iota_d = nc.dram_tensor("iota", [128], f32, kind="ExternalInput")
    niota_d = nc.dram_tensor("niota", [128], f32, kind="ExternalInput")
    out_d = nc.dram_tensor("out", [h_rows, D], f32, kind="ExternalOutput")

    CT = CHUNK_TILES

    with tile.TileContext(nc) as tc:
        with (
            tc.tile_pool(name="const", bufs=1) as constp,
            tc.tile_pool(name="data", bufs=7) as datap,
            tc.tile_pool(name="oh", bufs=6) as ohp,
            tc.tile_pool(name="small", bufs=6) as smallp,
            tc.tile_pool(name="fin", bufs=1) as finp,
            tc.tile_pool(name="dram", bufs=1, space="DRAM") as dramp,
            tc.tile_pool(name="psum", bufs=1, space="PSUM") as psump,
            tc.tile_pool(name="psz", bufs=3, space="PSUM") as pszp,
        ):
            tinyb = constp.tile([128, 1], f32)
            nc.vector.memset(tinyb[:], TINY)
            shiftb = constp.tile([128, 1], f32)
            nc.vector.memset(shiftb[:], -SHIFT)
            epsb = constp.tile([128, 1], f32)
            nc.vector.memset(epsb[:], EPS)
            ones = constp.tile([128, 1], f32)
            nc.vector.memset(ones[:], 1.0)
            iotat = constp.tile([128, 128], bf16)
            iotf32 = constp.tile([128, 128], f32)
            nc.sync.dma_start(iotf32[:], iota_d.ap().partition_broadcast(128))
            nc.vector.tensor_copy(iotat[:], iotf32[:])
            niotac = constp.tile([128, 1], f32)
            nc.sync.dma_start(niotac[:], niota_d.ap().unsqueeze(1))

            # preload all SWDGE indices + dstrel (partition-major layouts)
            isrc_sb = constp.tile([128, t_total, 8], i16)
            nc.sync.dma_start(isrc_sb[:], isrc_d[:, :, :])
            drel_sb = constp.tile([128, t_total], bf16)
            nc.sync.dma_start(drel_sb[:], drel_d[:, :])

            h_all = finp.tile([128, D, nw], f32)

            g = 0  # global tile cursor
            kq = 0  # chunk counter for SWDGE queue alternation
            for ph in plan["phases"]:
                nwp = len(ph)
                w0 = ph[0]
                psb = psump.tile([128, MAX_PSUM_WIN, 512], f32, tag="psb")
                zwin = datap.tile([128, MAX_PSUM_WIN, D], f32, tag="zwin")
                nc.sync.dma_start(
                    zwin[:, 0:nwp, :],
                    zs_d[w0 * WIN : (w0 + nwp) * WIN, :].rearrange(
                        "(w p) d -> p w d", p=128
                    ),
                )
                ph_secs = [
                    (ty, tl) for (ty, tl) in plan["sections"] if tl[0][0] in ph
                ]
                for ty, tl in ph_secs:
                    table = z_d[0:split, :] if ty == 0 else z_d[split:n_nodes, :]
                    for c0 in range(0, len(tl), CT):
                        ct = min(CT, len(tl) - c0)
                        t0 = g + c0
                        ne = ct * 128

                        zsrc = datap.tile([128, CT, D], f32, tag="zsrc")
                        nc.gpsimd.dma_gather(
                            zsrc[:, 0:ct, :],
                            table,
                            isrc_sb[:, t0 : t0 + ct, :],
                            ne,
                            ne,
                            D,
                            queue_num=kq % 4,
                        )
                        kq += 1

                        # transposed one-hot on the Scalar engine:
                        # ohT[n, e] = relu(1 - (drel_e - n)^2)
                        drbc = ohp.tile([128, CT * 128], bf16, tag="drbc")
                        nc.sync.dma_start(
                            drbc[:, 0:ne],
                            drelf_d[t0 * 128 : t0 * 128 + ne].partition_broadcast(
                                128
                            ),
                        )
                        ohT = ohp.tile([128, CT * 128], f32, tag="ohT")
                        nc.scalar.activation(
                            ohT[:, 0:ne], drbc[:, 0:ne], ACTF.Square,
                            bias=niotac[:], scale=1.0,
                        )
                        nc.scalar.activation(
                            ohT[:, 0:ne], ohT[:, 0:ne], ACTF.Relu,
                            bias=ones[:], scale=-1.0,
                        )

                        # z[dst] expansion: psum_zd[:, t, :] = ohT_t^T @ zwin_t
                        pzd = pszp.tile([128, CT, D], f32, tag="zd")
                        for tl_i in range(ct):
                            win = tile_meta[t0 + tl_i][0]
                            nc.tensor.matmul(
                                pzd[:, tl_i, :],
                                ohT[:, tl_i * 128 : (tl_i + 1) * 128],
                                zwin[:, win - w0, :],
                                start=tl_i == 0,
                                stop=tl_i == ct - 1,
                            )

                        # edge scores and weights
                        prod = datap.tile([128, CT, D], f32, tag="prod")
                        e = smallp.tile([128, CT], f32, tag="e")
                        wt = smallp.tile([128, CT], f32, tag="wt")
                        nc.vector.tensor_mul(
                            prod[:, 0:ct, :], zsrc[:, 0:ct, :], pzd[:, 0:ct, :]
                        )
                        nc.vector.tensor_reduce(
                            e[:, 0:ct], prod[:, 0:ct, :], axis=AX.X, op=ALU.add
                        )
                        nc.scalar.activation(
                            e[:, 0:ct], e[:, 0:ct], ACTF.Relu,
                            bias=0.0, scale=1.0,
                        )
                        nc.scalar.activation(
                            wt[:, 0:ct], e[:, 0:ct], ACTF.Exp,
                            bias=shiftb[:], scale=1.0,
                        )

                        # vals = [w * z_src | w] in bf16
                        vals = datap.tile([128, CT, D + 1], bf16, tag="vals")
                        nc.scalar.copy(vals[:, 0:ct, D], wt[:, 0:ct])
                        nc.vector.tensor_mul(
                            vals[:, 0:ct, 0:D],
                            zsrc[:, 0:ct, :],
                            wt[:, 0:ct].unsqueeze(2).broadcast_to((128, ct, D)),
                        )

                        # aggregation one-hot (edge-major) in bf16
                        oh = ohp.tile([128, CT, 128], bf16, tag="oh")
                        nc.vector.tensor_tensor(
                            oh[:, 0:ct, :],
                            iotat[:].unsqueeze(1).broadcast_to((128, ct, 128)),
                            drel_sb[:, t0 : t0 + ct]
                            .unsqueeze(2)
                            .broadcast_to((128, ct, 128)),
                            op=ALU.is_equal,
                        )

                        for tl_i in range(ct):
                            win, st, sp = tile_meta[t0 + tl_i]
                            slot = win - w0
                            nc.tensor.matmul(
                                psb[:, slot, 0 : D + 1],
                                oh[:, tl_i, :],
                                vals[:, tl_i, :],
                                start=st,
                                stop=sp,
                            )
                    g += len(tl)

                # drain phase: h = num / denom, written feature-major
                denp = smallp.tile([128, MAX_PSUM_WIN], f32, tag="den")
                recp = smallp.tile([128, MAX_PSUM_WIN], f32, tag="rec")
                nc.scalar.activation(
                    denp[:, 0:nwp], psb[:, 0:nwp, D], ACTF.Identity,
                    bias=tinyb[:], scale=1.0,
                )
                nc.vector.reciprocal(recp[:, 0:nwp], denp[:, 0:nwp])
                nc.vector.tensor_mul(
                    h_all[:, :, w0 : w0 + nwp].transpose((0, 2, 1)),
                    psb[:, 0:nwp, 0:D],
                    recp[:, 0:nwp].unsqueeze(2).broadcast_to((128, nwp, D)),
                )

            # ---- BatchNorm stats: s1 = sum(h), s2 = sum(h^2) over all nodes
            hsq = finp.tile([128, D, nw], f32, tag="hsq")
            nc.scalar.square(hsq[:], h_all[:])

            stats = smallp.tile([128, 2 * D], f32, tag="stats")
            nc.vector.tensor_reduce(
                stats[:, 0:D], h_all[:], axis=AX.X, op=ALU.add
            )
            nc.vector.tensor_reduce(
                stats[:, D : 2 * D], hsq[:], axis=AX.X, op=ALU.add
            )

            ps = pszp.tile([1, 2 * D], f32, tag="zd")
            nc.tensor.matmul(ps[:], ones[:], stats[:], start=True, stop=True)
            srow = smallp.tile([1, 2 * D], f32, tag="srow")
            nc.scalar.copy(srow[:], ps[:])

            cc_in = dramp.tile([1, 2 * D], f32)
            cc_out = dramp.tile([1, 2 * D], f32)
            nc.sync.dma_start(cc_in[:], srow[:])
            nc.gpsimd.collective_compute(
                "AllReduce",
                ALU.add,
                ins=[cc_in.opt()],
                outs=[cc_out.opt()],
                replica_groups=[list(range(NCORES))],
            )

            G = smallp.tile([128, 2 * D], f32, tag="G")
            nc.sync.dma_start(G[:], cc_out[:].squeeze(0).partition_broadcast(128))
            gbB = constp.tile([128, 2 * D], f32)
            nc.sync.dma_start(gbB[:], gb_d.ap().flatten().partition_broadcast(128))

            inv_n = 1.0 / float(n_total_nodes)
            mean = smallp.tile([128, D], f32, tag="mean")
            var = smallp.tile([128, D], f32, tag="var")
            nc.scalar.mul(mean[:], G[:, 0:D], inv_n)
            nc.scalar.mul(var[:], G[:, D : 2 * D], inv_n)
            msq = smallp.tile([128, D], f32, tag="msq")
            nc.vector.tensor_mul(msq[:], mean[:], mean[:])
            nc.vector.tensor_sub(var[:], var[:], msq[:])
            std = smallp.tile([128, D], f32, tag="std")
            nc.scalar.activation(std[:], var[:], ACTF.Sqrt, bias=epsb[:], scale=1.0)
            rstd = smallp.tile([128, D], f32, tag="rstd")
            nc.vector.reciprocal(rstd[:], std[:])

            a = smallp.tile([128, D], f32, tag="a")
            b = smallp.tile([128, D], f32, tag="b")
            nc.vector.tensor_mul(a[:], gbB[:, 0:D], rstd[:])
            nc.vector.tensor_mul(b[:], mean[:], a[:])
            nc.vector.tensor_sub(b[:], gbB[:, D : 2 * D], b[:])

            # y stored node-major so the output DMA gets 256B-contiguous runs
            y = finp.tile([128, nw, D], f32, tag="hsq")
            nc.vector.tensor_mul(
                y[:],
                h_all[:].transpose((0, 2, 1)),
                a[:].unsqueeze(1).broadcast_to((128, nw, D)),
            )
            nc.vector.tensor_add(
                y[:], y[:], b[:].unsqueeze(1).broadcast_to((128, nw, D))
            )
            nc.vector.tensor_relu(y[:], y[:])

            outv = out_d.ap().rearrange("(c p) f -> p c f", p=128)
            nc.sync.dma_start(outv, y[:])

    nc.compile()
    return nc


# ---------------------------------------------------------------- entry point
TRACE = False          # set True by test harnesses to capture exec_time_ns
LAST_RESULT = None     # BassKernelResults of the most recent kernel() call


def kernel(**inputs):
    z = inputs["z"]
    src = inputs["src"]
    dst = inputs["dst"]
    gamma = inputs["gamma"]
    beta = inputs["beta"]

    from concourse.bass_utils import run_bass_kernel_spmd

    in_maps, plan = prep_inputs(z, src, dst, gamma, beta)
    nc = build_nc(plan)
    res = run_bass_kernel_spmd(
        nc, in_maps, core_ids=list(range(NCORES)), trace=TRACE
    )
    global LAST_RESULT
    LAST_RESULT = res

    npc = CFG["npc"]
    out = np.empty((N_NODES, D), dtype=np.float32)
    for c in range(NCORES):
        out[c * npc : (c + 1) * npc] = res.results[c]["out"][:npc]
    return out
